# revision 11
# baseline (speedup 1.0000x reference)
"""Distributed spectral conv on S2 (SHT -> per-l complex channel mix -> ISHT)
for Trainium2, m-mode sharded across 8 NeuronCores (strided: core r gets
m = r, r+8, r+16, ... so every core's m-list has uniform parity).

v2 pipeline per core (33 m-slots, MC=66 real components):
  A2: DFT as matmul, x-chunks stationary -> psum [ck 128, cm 66] (k on
      partitions directly; host bakes k-reversal for the second k-half)
  F:  parity fold XE/XO = kh0 +- kh1 (psum-pair -> sbuf bf16)
  B:  parity Legendre, K=128 bf16: per m two matmuls (XE@shtw+, XO@shtw-)
  C:  per-l-pair channel mix, block-diag built on-chip from compact weights,
      2 matmuls per l using a negated-cfi copy in CFQ3
  P1: PE-transpose COUT4 -> OUTT [l2, (blk, m, cp, o)]
  D:  parity inverse Legendre + psum add/sub fold -> XKS (kc=1 k-reversed)
  P2: PE-transpose -> XK [(m,comp), (o,k')]
  E:  inverse DFT bf16 (contract m-comps) -> y_part [CK, NLON]
Host sums the 8 partial y outputs and un-reverses the second k-half.
"""
import numpy as np
import ml_dtypes

import concourse.bass as bass
import concourse.bacc as bacc
import concourse.mybir as mybir
from concourse import tile
from concourse._compat import get_trn_type
from concourse.bass_utils import run_bass_kernel_spmd

F32 = mybir.dt.float32
BF16 = mybir.dt.bfloat16
ADD = mybir.AluOpType.add
SUB = mybir.AluOpType.subtract

N_CORES = 8
M_LOC = 33            # m slots per core (core 0: 33 real m's, others 32+pad)
MC = 2 * M_LOC
CIN = 64
COUT_ = 64
NLAT = 256
NLON = 512
MMAX = 257
CK = COUT_ * NLAT

GP_PSUM = False       # TRN2: GPSIMD instructions cannot access PSUM

_prog_cache = {}


def _build_nc(stages="ABCDE"):
    nc = bacc.Bacc(get_trn_type() or "TRN2", target_bir_lowering=False, debug=False)

    xt = nc.dram_tensor("xt", [4, 128, CK], BF16, kind="ExternalInput")
    fdft = nc.dram_tensor("fdft", [4, 128, MC], BF16, kind="ExternalInput")
    shtw = nc.dram_tensor("shtw", [128, M_LOC, 256], BF16, kind="ExternalInput")
    wc = nc.dram_tensor("wc", [128, 128, 2, 64], BF16, kind="ExternalInput")
    pctb = nc.dram_tensor("pctb", [128, M_LOC, 384], BF16, kind="ExternalInput")
    gdft = nc.dram_tensor("gdft", [MC, NLON], BF16, kind="ExternalInput")
    ident = nc.dram_tensor("ident", [128, 128], BF16, kind="ExternalInput")
    y_part = nc.dram_tensor("y_part", [CK, NLON], BF16, kind="ExternalOutput")

    with tile.TileContext(nc) as tc:
        with tc.tile_pool(name="const", bufs=1) as constp, \
             tc.tile_pool(name="big", bufs=1) as bigp, \
             tc.tile_pool(name="xa", bufs=2) as xap, \
             tc.tile_pool(name="sw", bufs=3) as swp, \
             tc.tile_pool(name="wcp", bufs=2) as wcp, \
             tc.tile_pool(name="wt", bufs=2) as wtp, \
             tc.tile_pool(name="pt", bufs=3) as ptp, \
             tc.tile_pool(name="ysb", bufs=4) as ysbp, \
             tc.tile_pool(name="ps", bufs=4, space="PSUM") as psp, \
             tc.tile_pool(name="pst", bufs=4, space="PSUM") as pstp:

            xar0 = xap.tile([128, 4, 1024], BF16, tag="xar")
            nc.sync.dma_start(
                xar0[:, :, :],
                xt.ap()[:, :, 0:1024].rearrange("a b c -> b a c")
            )
            fsbr = constp.tile([128, 4, MC], BF16)      # [n_in_chunk, nchunk, cm]
            fsbrN = constp.tile([128, 4, MC], BF16)     # negated (for XO fold)
            gsb = constp.tile([MC, NLON], BF16)
            isbb = constp.tile([128, 128], BF16)
            nc.sync.dma_start(fsbr[:, :, :], fdft.ap().rearrange("a b c -> b a c"))
            nc.sync.dma_start(gsb[:, :], gdft[:, :])
            nc.sync.dma_start(isbb[:, :], ident[:, :])
            nc.vector.tensor_scalar_mul(fsbrN[:, :, :], fsbr[:, :, :], -1.0)

            # ---- stage A2 + parity fold in PE accumulation ----
            # XE/XO [128 ki, (mi, comp, c)]: XE = DFT(kh0) + DFT(kh1),
            # XO = DFT(kh0) - DFT(kh1)  (kh1 k-reversal baked in xt on host)
            XE = bigp.tile([128, M_LOC * 2 * CIN], BF16, tag="bigE")
            XO = bigp.tile([128, M_LOC * 2 * CIN], BF16, tag="bigO")
            XE_v = XE.rearrange("p (mi comp c) -> p mi comp c", comp=2, c=CIN)
            XO_v = XO.rearrange("p (mi comp c) -> p mi comp c", comp=2, c=CIN)
            for span in range(16):          # 1024 ck-columns per span
                if span == 0:
                    xar = xar0
                else:
                    xar = xap.tile([128, 4, 1024], BF16, tag="xar")
                    nc.sync.dma_start(
                        xar[:, :, :],
                        xt.ap()[:, :, span * 1024:(span + 1) * 1024]
                        .rearrange("a b c -> b a c")
                    )
                for half in range(2):       # 2 channels x (E,O) per psum
                    pa = psp.tile([128, 2, 2, MC], F32, tag="ps")
                    for cc in range(2):
                        c_loc = half * 2 + cc
                        for eo in range(2):
                            for kh in range(2):
                                rhs = fsbr if (eo == 0 or kh == 0) else fsbrN
                                for nc4 in range(4):
                                    nc.tensor.matmul(
                                        pa[:, cc, eo, :],
                                        xar[:, nc4, c_loc * 256 + kh * 128:
                                            c_loc * 256 + kh * 128 + 128],
                                        rhs[:, nc4, :],
                                        start=(cc == 0 and eo == 0 and kh == 0 and nc4 == 0),
                                        stop=(cc == 1 and eo == 1 and kh == 1 and nc4 == 3),
                                    )
                    c0 = span * 4 + half * 2
                    srcE = pa[:, :, 0, :].rearrange("p cc (comp mi) -> p mi comp cc", comp=2)
                    srcO = pa[:, :, 1, :].rearrange("p cc (comp mi) -> p mi comp cc", comp=2)
                    if half == 0:
                        nc.vector.tensor_copy(XE_v[:, :, :, c0:c0 + 2], srcE)
                        nc.scalar.copy(XO_v[:, :, :, c0:c0 + 2], srcO)
                    else:
                        nc.scalar.copy(XE_v[:, :, :, c0:c0 + 2], srcE)
                        nc.vector.tensor_copy(XO_v[:, :, :, c0:c0 + 2], srcO)

            if "B" not in stages:
                dbg = ysbp.tile([128, NLON], BF16, tag="ys")
                nc.vector.tensor_copy(dbg[:, :], XE[:, 0:NLON])
                nc.sync.dma_start(y_part[0:128, :], dbg[:, :])
                nc.vector.tensor_copy(dbg[:, :], XO[:, 0:NLON])
                nc.sync.dma_start(y_part[128:256, :], dbg[:, :])
                return nc

            # ---- stage B: parity Legendre bf16 -> CFQ3 ----
            # CFQ3 [(blk,i) 128, (cm3, j)]; cm3: [0:33]=-cfi, [33:66]=cfr, [66:99]=cfi
            CFQ3 = bigp.tile([128, 99 * 128], BF16, tag="bigB")
            CFQ3_v = CFQ3.rearrange("p (cm j) -> p cm j", j=128)
            for mib in range(0, M_LOC, 2):
                nm2 = min(2, M_LOC - mib)
                if mib % 4 == 0:
                    nmw = min(4, M_LOC - mib)
                    swr = swp.tile([128, 4, 256], BF16, tag="sw")
                    nc.sync.dma_start(
                        swr[:, 0:nmw, :], shtw.ap()[:, mib:mib + nmw, :]
                    )
                pb = psp.tile([128, nm2 * 256], F32, tag="ps")
                pb_v = pb.rearrange("p (m2 blk j) -> p m2 blk j", blk=2, j=128)
                for m2 in range(nm2):
                    mi = mib + m2
                    nc.tensor.matmul(
                        pb_v[:, m2, 0, :],
                        XE[:, mi * 128:(mi + 1) * 128],
                        swr[:, (mib % 4) + m2, 0:128],
                        start=(m2 == 0), stop=False,
                    )
                    nc.tensor.matmul(
                        pb_v[:, m2, 1, :],
                        XO[:, mi * 128:(mi + 1) * 128],
                        swr[:, (mib % 4) + m2, 128:256],
                        start=False, stop=(m2 == nm2 - 1),
                    )
                for blk in range(2):
                    for comp in range(2):
                        dst = CFQ3_v[blk * 64:(blk + 1) * 64,
                                     (33 if comp == 0 else 66) + mib:
                                     (33 if comp == 0 else 66) + mib + nm2, :]
                        src = pb_v[comp * 64:(comp + 1) * 64, :, blk, :]
                        if (mib // 2 + blk) % 2 == 0:
                            nc.vector.tensor_copy(dst, src)
                        else:
                            nc.scalar.copy(dst, src)
            # bulk negate: cm3[0:33] = -cfi
            nc.gpsimd.tensor_scalar_mul(
                CFQ3_v[:, 0:33, :], CFQ3_v[:, 66:99, :], -1.0
            )

            if "C" not in stages:
                dbg = ysbp.tile([128, NLON], BF16, tag="ys")
                nc.vector.tensor_copy(dbg[:, :], CFQ3[:, 0:NLON])
                nc.sync.dma_start(y_part[0:128, :], dbg[:, :])
                return nc

            # ---- stage C: channel mix, block-diag built on-chip, 2 MMs/l ----
            COUT4 = bigp.tile([64, 66 * 256], BF16, tag="bigA")
            COUT4_v = COUT4.rearrange("p (cm l) -> p cm l", l=256)
            for ci in range(8):
                wcT = wcp.tile([128, 16, 2, 64], BF16, tag="wc")
                nc.sync.dma_start(
                    wcT[:, :, :, :], wc.ap()[:, ci * 16:(ci + 1) * 16, :, :]
                )
                wt = wtp.tile([128, 16, 2, 128], BF16, tag="wt")
                if ci < 2:
                    nc.gpsimd.memset(wt[:, :, :, :], 0.0)
                nc.gpsimd.tensor_copy(wt[0:64, :, :, 0:64], wcT[0:64, :, :, :])
                nc.gpsimd.tensor_copy(wt[64:128, :, :, 64:128], wcT[64:128, :, :, :])
                for jj in range(0, 16, 2):
                    j = ci * 16 + jj
                    pc = psp.tile([128, 2, MC], F32, tag="ps")
                    for h in range(2):
                        nc.tensor.matmul(pc[:, h, :], wt[:, jj + h, 0, :],
                                         CFQ3_v[:, 33:99, j + h],
                                         start=(h == 0), stop=False)
                        nc.tensor.matmul(pc[:, h, :], wt[:, jj + h, 1, :],
                                         CFQ3_v[:, 0:66, j + h],
                                         start=False, stop=(h == 1))
                    d0 = COUT4_v[:, :, j:j + 2].rearrange("p cm h -> p h cm")
                    d1 = COUT4_v[:, :, 128 + j:130 + j].rearrange("p cm h -> p h cm")
                    if (j // 2) % 2 == 0:
                        nc.vector.tensor_copy(d0, pc[0:64, :, :])
                        nc.scalar.copy(d1, pc[64:128, :, :])
                    else:
                        nc.scalar.copy(d0, pc[0:64, :, :])
                        nc.vector.tensor_copy(d1, pc[64:128, :, :])

            if "P1" not in stages and "D" not in stages:
                dbg = ysbp.tile([64, NLON], BF16, tag="ys")
                nc.vector.tensor_copy(dbg[:, :], COUT4[:, 0:NLON])
                nc.sync.dma_start(y_part[0:64, :], dbg[:, :])
                return nc

            # ---- pivot P1: COUT4 -> OUTT [l2, (blk, mi, cp, o)] ----
            OUTT = bigp.tile([128, 2 * M_LOC * 2 * 64], BF16, tag="bigC")
            OUTT_v = OUTT.rearrange("p (blk mi cp o) -> p blk mi cp o",
                                    blk=2, cp=2, o=64)
            for cm in range(MC):
                cp, mi = cm // M_LOC, cm % M_LOC
                ptr = pstp.tile([128, 2, 64], BF16, tag="pst")
                for blk in range(2):
                    nc.tensor.transpose(
                        ptr[:, blk, :],
                        COUT4_v[:, cm, blk * 128:(blk + 1) * 128], isbb[:64, :64]
                    )
                dst = OUTT_v[:, :, mi, cp, :]
                if cm % 2 == 0:
                    nc.vector.tensor_copy(dst, ptr[:, :, :])
                else:
                    nc.scalar.copy(dst, ptr[:, :, :])

            if "D" not in stages:
                dbg = ysbp.tile([128, NLON], BF16, tag="ys")
                nc.vector.tensor_copy(dbg[:, :], OUTT[:, 0:NLON])
                nc.sync.dma_start(y_part[0:128, :], dbg[:, :])
                return nc

            # ---- stage D: parity inverse Legendre, k-fold in PE accum ----
            # XKS [128 ki, (kc 2, o 64, cm 66)]; kc=1 rows are k-reversed
            # pcs blocks: [0:128]=pct(blk0), [128:256]=pct(blk1), [256:384]=-pct(blk1)
            XKS = bigp.tile([128, 2 * 64 * MC], BF16, tag="bigA2")
            XKS_v = XKS.rearrange("p (kc o cm) -> p kc o cm", kc=2, o=64)
            for mib in range(0, M_LOC, 2):
                nm2 = min(2, M_LOC - mib)
                if mib % 4 == 0:
                    nmw = min(4, M_LOC - mib)
                    pcs = ptp.tile([128, 4, 384], BF16, tag="pt")
                    nc.sync.dma_start(
                        pcs[:, 0:nmw, :], pctb.ap()[:, mib:mib + nmw, :]
                    )
                pd = psp.tile([128, nm2, 2, 128], F32, tag="ps")
                for m2 in range(nm2):
                    mi = mib + m2
                    ow = (mib % 4) + m2
                    r0 = OUTT_v[:, 0, mi, :, :].rearrange("p cp o -> p (cp o)")
                    r1 = OUTT_v[:, 1, mi, :, :].rearrange("p cp o -> p (cp o)")
                    first = (m2 == 0)
                    last = (m2 == nm2 - 1)
                    # kc0 = blk0 + blk1 ; kc1 = blk0 - blk1
                    nc.tensor.matmul(pd[:, m2, 0, :], pcs[:, ow, 0:128], r0,
                                     start=first, stop=False)
                    nc.tensor.matmul(pd[:, m2, 0, :], pcs[:, ow, 128:256], r1,
                                     start=False, stop=False)
                    nc.tensor.matmul(pd[:, m2, 1, :], pcs[:, ow, 0:128], r0,
                                     start=False, stop=False)
                    nc.tensor.matmul(pd[:, m2, 1, :], pcs[:, ow, 256:384], r1,
                                     start=False, stop=last)
                for kc in range(2):
                    src = pd[:, :, kc, :].rearrange("p m2 (cp o) -> p m2 cp o",
                                                    cp=2, o=64)
                    dst = XKS_v[:, kc, :, 2 * mib:2 * (mib + nm2)] \
                        .rearrange("p o (m2 cp) -> p m2 cp o", cp=2)
                    if (mib // 2 + kc) % 2 == 0:
                        nc.vector.tensor_copy(dst, src)
                    else:
                        nc.scalar.copy(dst, src)

            if "P2" not in stages and "E" not in stages:
                dbg = ysbp.tile([128, NLON], BF16, tag="ys")
                nc.vector.tensor_copy(dbg[:, :], XKS[:, 0:NLON])
                nc.sync.dma_start(y_part[0:128, :], dbg[:, :])
                return nc

            # ---- pivot P2: XKS -> XK [(m,comp), (o, kpos)] ----
            XK = bigp.tile([MC, CK], BF16, tag="bigB2")
            XK_v = XK.rearrange("p (o k) -> p o k", k=NLAT)
            for o in range(64):
                pt2 = pstp.tile([MC, 2, 128], BF16, tag="pst")
                for kc in range(2):
                    nc.tensor.transpose(
                        pt2[:, kc, :], XKS_v[:, kc, o, :], isbb[:, :]
                    )
                if o % 2 == 0:
                    nc.vector.tensor_copy(XK_v[:, o, :], pt2.rearrange("p a b -> p (a b)"))
                else:
                    nc.scalar.copy(XK_v[:, o, :], pt2.rearrange("p a b -> p (a b)"))

            if "E" not in stages:
                dbg = ysbp.tile([MC, NLON], BF16, tag="ys")
                nc.vector.tensor_copy(dbg[:, :], XK[0:MC, 0:NLON])
                nc.sync.dma_start(y_part[0:MC, :], dbg[:, :])
                return nc

            # ---- stage E: inverse DFT as matmul bf16, contract m-comps ----
            nmod = 3 if GP_PSUM else 2
            for jp in range(CK // 256):
                ys = ysbp.tile([128, 2, NLON], BF16, tag="ys2")
                for h in range(2):
                    j = 2 * jp + h
                    pe = psp.tile([128, NLON], F32, tag="ps")
                    nc.tensor.matmul(
                        pe[:, :], XK[:, j * 128:(j + 1) * 128], gsb[:, :],
                        start=True, stop=True,
                    )
                    sel = (2 * jp + h) % nmod
                    if sel == 0:
                        nc.scalar.copy(ys[:, h, :], pe[:, :])
                    elif sel == 1:
                        nc.vector.tensor_copy(ys[:, h, :], pe[:, :])
                    else:
                        nc.gpsimd.tensor_copy(ys[:, h, :], pe[:, :])
                nc.sync.dma_start(
                    y_part.ap()[jp * 256:(jp + 1) * 256, :]
                    .rearrange("(a p) n -> p a n", a=2),
                    ys[:, :, :],
                )

    return nc


def _get_nc(stages="ABCDE"):
    if stages not in _prog_cache:
        nc = _build_nc(stages)
        nc.compile()
        _prog_cache[stages] = nc
    return _prog_cache[stages]


def _core_ms(r):
    return [8 * j + r for j in range(M_LOC) if 8 * j + r < MMAX]


def make_in_maps(x, weight_r, weight_i, pct, sht_w):
    x = np.asarray(x, dtype=np.float32)
    wr = np.asarray(weight_r, dtype=np.float32)[0]          # [i, o, l]
    wi = np.asarray(weight_i, dtype=np.float32)[0]
    pct = np.asarray(pct, dtype=np.float32)                 # [m, l, k]
    sht_w = np.asarray(sht_w, dtype=np.float32)

    # xt[nc4, ni, ck], ck = c*256 + kidx; kidx>=128 holds k = 255-(kidx-128)
    x2 = x[0].reshape(CIN, NLAT, NLON)
    xr = np.concatenate([x2[:, :128], x2[:, 255:127:-1]], axis=1)
    xt = np.ascontiguousarray(
        xr.reshape(CK, NLON).T.reshape(4, 128, CK)
    ).astype(ml_dtypes.bfloat16)
    ident = np.eye(128, dtype=np.float32).astype(ml_dtypes.bfloat16)

    n = np.arange(NLON)
    j128 = np.arange(128)
    in_maps = []
    wc_cache = {}
    for r in range(N_CORES):
        ms = _core_ms(r)
        nm = len(ms)
        marr = np.array(ms)
        p_core = r % 2

        ang = 2.0 * np.pi * marr[None, :] * n[:, None] / NLON   # [n, nm]
        fdft = np.zeros((NLON, MC), np.float32)
        fdft[:, :nm] = (2.0 * np.pi / NLON) * np.cos(ang)
        fdft[:, M_LOC:M_LOC + nm] = -(2.0 * np.pi / NLON) * np.sin(ang)
        fdft = fdft.reshape(4, 128, MC)

        cmf = np.where((marr == 0) | (marr == NLON // 2), 1.0, 2.0)
        gdft = np.zeros((MC, NLON), np.float32)   # rows interleaved (m, comp)
        gdft[0:2 * nm:2, :] = cmf[:, None] * np.cos(ang.T)
        gdft[1:2 * nm:2, :] = -cmf[:, None] * np.sin(ang.T)

        l_p0 = 2 * j128 + p_core          # sigma+ l's
        l_p1 = 2 * j128 + 1 - p_core      # sigma- l's

        shtw_h = np.zeros((128, M_LOC, 256), np.float32)   # [ki, mi, (blk,j)]
        pct_h = np.zeros((128, M_LOC, 384), np.float32)    # [l2, mi, (b0,b1,-b1)]
        for mi, m in enumerate(ms):
            shtw_h[:, mi, 0:128] = sht_w[m][l_p0, :128].T
            shtw_h[:, mi, 128:256] = sht_w[m][l_p1, :128].T
            pct_h[:, mi, 0:128] = pct[m][l_p0, :128]
            pct_h[:, mi, 128:256] = pct[m][l_p1, :128]
            pct_h[:, mi, 256:384] = -pct[m][l_p1, :128]

        if p_core not in wc_cache:
            wca = np.zeros((128, 128, 2, 64), np.float32)
            wca[0:64, :, 0, :] = wr[:, :, l_p0].transpose(0, 2, 1)
            wca[64:128, :, 0, :] = wr[:, :, l_p1].transpose(0, 2, 1)
            wca[0:64, :, 1, :] = wi[:, :, l_p0].transpose(0, 2, 1)
            wca[64:128, :, 1, :] = wi[:, :, l_p1].transpose(0, 2, 1)
            wc_cache[p_core] = np.ascontiguousarray(wca).astype(ml_dtypes.bfloat16)

        in_maps.append({
            "xt": xt,
            "fdft": np.ascontiguousarray(fdft).astype(ml_dtypes.bfloat16),
            "shtw": shtw_h.astype(ml_dtypes.bfloat16),
            "wc": wc_cache[p_core],
            "pctb": pct_h.astype(ml_dtypes.bfloat16),
            "gdft": gdft.astype(ml_dtypes.bfloat16),
            "ident": ident,
        })
    return in_maps


def kernel(x, weight_r, weight_i, pct, sht_w):
    x_np = np.asarray(x)
    nc = _get_nc()
    in_maps = make_in_maps(x_np, weight_r, weight_i, pct, sht_w)
    try:
        res = run_bass_kernel_spmd(nc, in_maps, list(range(N_CORES)))
    except Exception:
        # transient NRT exec faults have been observed on the first run
        # after a NEFF load; one retry has always succeeded
        res = run_bass_kernel_spmd(nc, in_maps, list(range(N_CORES)))
    y = np.zeros((CK, NLON), np.float64)
    for r in range(N_CORES):
        y += np.asarray(res.results[r]["y_part"], dtype=np.float64)
    y4 = y.astype(np.float32).reshape(COUT_, 2, 128, NLON)
    yf = np.concatenate([y4[:, 0], y4[:, 1, ::-1]], axis=1)
    return (yf.reshape(1, COUT_, NLAT, NLON), x_np)


# revision 15
# speedup vs baseline: 1.2613x; 1.2613x over previous
"""Distributed spectral conv on S2 (SHT -> per-l complex channel mix -> ISHT)
for Trainium2, m-mode sharded across 8 NeuronCores (strided: core r gets
m = r, r+8, r+16, ... so every core's m-list has uniform parity).

v2 pipeline per core (33 m-slots, MC=66 real components):
  A2: DFT as matmul, x-chunks stationary -> psum [ck 128, cm 66] (k on
      partitions directly; host bakes k-reversal for the second k-half)
  F:  parity fold XE/XO = kh0 +- kh1 (psum-pair -> sbuf bf16)
  B:  parity Legendre, K=128 bf16: per m two matmuls (XE@shtw+, XO@shtw-)
  C:  per-l-pair channel mix, block-diag built on-chip from compact weights,
      2 matmuls per l using a negated-cfi copy in CFQ3
  P1: PE-transpose COUT4 -> OUTT [l2, (blk, m, cp, o)]
  D:  parity inverse Legendre + psum add/sub fold -> XKS (kc=1 k-reversed)
  P2: PE-transpose -> XK [(m,comp), (o,k')]
  E:  inverse DFT bf16 (contract m-comps) -> y_part [CK, NLON]
Host sums the 8 partial y outputs and un-reverses the second k-half.
"""
import numpy as np
import ml_dtypes

import concourse.bass as bass
import concourse.bacc as bacc
import concourse.mybir as mybir
from concourse import tile
from concourse._compat import get_trn_type
from concourse.bass_utils import run_bass_kernel_spmd

F32 = mybir.dt.float32
BF16 = mybir.dt.bfloat16
ADD = mybir.AluOpType.add
SUB = mybir.AluOpType.subtract

N_CORES = 8
M_LOC = 33            # m slots per core (core 0: 33 real m's, others 32+pad)
MC = 2 * M_LOC
CIN = 64
COUT_ = 64
NLAT = 256
NLON = 512
MMAX = 257
CK = COUT_ * NLAT

GP_PSUM = False       # TRN2: GPSIMD instructions cannot access PSUM

_prog_cache = {}


def _build_nc(stages="ABCDE"):
    nc = bacc.Bacc(get_trn_type() or "TRN2", target_bir_lowering=False, debug=False)

    xt = nc.dram_tensor("xt", [4, 128, CK], BF16, kind="ExternalInput")
    fdft = nc.dram_tensor("fdft", [4, 128, MC], BF16, kind="ExternalInput")
    shtw = nc.dram_tensor("shtw", [128, M_LOC, 256], BF16, kind="ExternalInput")
    wc = nc.dram_tensor("wc", [128, 128, 2, 64], BF16, kind="ExternalInput")
    pctb = nc.dram_tensor("pctb", [128, M_LOC, 384], BF16, kind="ExternalInput")
    gdft = nc.dram_tensor("gdft", [MC, NLON], BF16, kind="ExternalInput")
    ident = nc.dram_tensor("ident", [128, 128], BF16, kind="ExternalInput")
    y_part = nc.dram_tensor("y_part", [CK, NLON], BF16, kind="ExternalOutput")

    with tile.TileContext(nc) as tc:
        with tc.tile_pool(name="const", bufs=1) as constp, \
             tc.tile_pool(name="big", bufs=1) as bigp, \
             tc.tile_pool(name="xa", bufs=2) as xap, \
             tc.tile_pool(name="sw", bufs=3) as swp, \
             tc.tile_pool(name="wcp", bufs=2) as wcp, \
             tc.tile_pool(name="wt", bufs=2) as wtp, \
             tc.tile_pool(name="pt", bufs=3) as ptp, \
             tc.tile_pool(name="ysb", bufs=4) as ysbp, \
             tc.tile_pool(name="ps", bufs=4, space="PSUM") as psp, \
             tc.tile_pool(name="pst", bufs=4, space="PSUM") as pstp:

            xar0 = xap.tile([128, 4, 1024], BF16, tag="xar")
            nc.sync.dma_start(
                xar0[:, :, :],
                xt.ap()[:, :, 0:1024].rearrange("a b c -> b a c")
            )
            fsbr = constp.tile([128, 4, MC], BF16)      # [n_in_chunk, nchunk, cm]
            fsbrN = constp.tile([128, 4, MC], BF16)     # negated (for XO fold)
            gsb = constp.tile([MC, NLON], BF16)
            isbb = constp.tile([128, 128], BF16)
            nc.sync.dma_start(fsbr[:, :, :], fdft.ap().rearrange("a b c -> b a c"))
            nc.sync.dma_start(gsb[:, :], gdft[:, :])
            nc.sync.dma_start(isbb[:, :], ident[:, :])
            nc.vector.tensor_scalar_mul(fsbrN[:, :, :], fsbr[:, :, :], -1.0)

            # ---- stage A: DFT as matmul, basis stationary (big-N MMs), then
            # T1 PE-transpose per channel and SBUF parity fold -> XE/XO
            # XE/XO [128 ki, (mi, comp, c)]: XE = xf(k) + xf(255-k) (kh1
            # k-reversal baked in xt on host)
            XE = bigp.tile([128, M_LOC * 2 * CIN], BF16, tag="bigE")
            XO = bigp.tile([128, M_LOC * 2 * CIN], BF16, tag="bigO")
            XE_v = XE.rearrange("p (mi comp c) -> p mi comp c", comp=2, c=CIN)
            XO_v = XO.rearrange("p (mi comp c) -> p mi comp c", comp=2, c=CIN)
            for span in range(16):          # 1024 ck-columns per span
                if span == 0:
                    xar = xar0
                else:
                    xar = xap.tile([128, 4, 1024], BF16, tag="xar")
                    nc.sync.dma_start(
                        xar[:, :, :],
                        xt.ap()[:, :, span * 1024:(span + 1) * 1024]
                        .rearrange("a b c -> b a c")
                    )
                for half in range(2):       # 512 ck-cols = 2 channels per psum
                    pa = psp.tile([MC, 512], F32, tag="ps")
                    for nc4 in range(4):
                        nc.tensor.matmul(
                            pa[:, :],
                            fsbr[:, nc4, :],
                            xar[:, nc4, half * 512:(half + 1) * 512],
                            start=(nc4 == 0),
                            stop=(nc4 == 3),
                        )
                    xf2 = xap.tile([MC, 512], BF16, tag="xf2")
                    nc.scalar.copy(xf2[:, :], pa[:, :])
                    for cc in range(2):     # channel c0+cc: q-blocks (kh0,kh1)
                        c = span * 4 + half * 2 + cc
                        ptr = pstp.tile([128, 2, MC], BF16, tag="pst")
                        for kh in range(2):
                            nc.tensor.transpose(
                                ptr[:, kh, :],
                                xf2[:, (2 * cc + kh) * 128:(2 * cc + kh + 1) * 128],
                                isbb[:MC, :MC]
                            )
                        xfc = xap.tile([128, 2, MC], BF16, tag="xfc")
                        if cc == 0:
                            nc.vector.tensor_copy(xfc[:, :, :], ptr[:, :, :])
                        else:
                            nc.scalar.copy(xfc[:, :, :], ptr[:, :, :])
                        s0 = xfc[:, 0, :].rearrange("p (comp mi) -> p mi comp", comp=2)
                        s1 = xfc[:, 1, :].rearrange("p (comp mi) -> p mi comp", comp=2)
                        nc.vector.tensor_tensor(XE_v[:, :, :, c], s0, s1, op=ADD)
                        nc.vector.tensor_tensor(XO_v[:, :, :, c], s0, s1, op=SUB)

            if "B" not in stages:
                dbg = ysbp.tile([128, NLON], BF16, tag="ys")
                nc.vector.tensor_copy(dbg[:, :], XE[:, 0:NLON])
                nc.sync.dma_start(y_part[0:128, :], dbg[:, :])
                nc.vector.tensor_copy(dbg[:, :], XO[:, 0:NLON])
                nc.sync.dma_start(y_part[128:256, :], dbg[:, :])
                return nc

            # ---- stage B: parity Legendre bf16 -> CFQ3 ----
            # CFQ3 [(blk,i) 128, (cm3, j)]; cm3: [0:33]=-cfi, [33:66]=cfr, [66:99]=cfi
            CFQ3 = bigp.tile([128, 99 * 128], BF16, tag="bigB")
            CFQ3_v = CFQ3.rearrange("p (cm j) -> p cm j", j=128)
            for mib in range(0, M_LOC, 2):
                nm2 = min(2, M_LOC - mib)
                if mib % 4 == 0:
                    nmw = min(4, M_LOC - mib)
                    swr = swp.tile([128, 4, 256], BF16, tag="sw")
                    nc.sync.dma_start(
                        swr[:, 0:nmw, :], shtw.ap()[:, mib:mib + nmw, :]
                    )
                pb = psp.tile([128, nm2 * 256], F32, tag="ps")
                pb_v = pb.rearrange("p (m2 blk j) -> p m2 blk j", blk=2, j=128)
                for m2 in range(nm2):
                    mi = mib + m2
                    nc.tensor.matmul(
                        pb_v[:, m2, 0, :],
                        XE[:, mi * 128:(mi + 1) * 128],
                        swr[:, (mib % 4) + m2, 0:128],
                        start=(m2 == 0), stop=False,
                    )
                    nc.tensor.matmul(
                        pb_v[:, m2, 1, :],
                        XO[:, mi * 128:(mi + 1) * 128],
                        swr[:, (mib % 4) + m2, 128:256],
                        start=False, stop=(m2 == nm2 - 1),
                    )
                for blk in range(2):
                    for comp in range(2):
                        dst = CFQ3_v[blk * 64:(blk + 1) * 64,
                                     (33 if comp == 0 else 66) + mib:
                                     (33 if comp == 0 else 66) + mib + nm2, :]
                        src = pb_v[comp * 64:(comp + 1) * 64, :, blk, :]
                        if (mib // 2 + blk) % 2 == 0:
                            nc.vector.tensor_copy(dst, src)
                        else:
                            nc.scalar.copy(dst, src)
            # bulk negate: cm3[0:33] = -cfi
            nc.vector.tensor_scalar_mul(
                CFQ3_v[:, 0:33, :], CFQ3_v[:, 66:99, :], -1.0
            )

            if "C" not in stages:
                dbg = ysbp.tile([128, NLON], BF16, tag="ys")
                nc.vector.tensor_copy(dbg[:, :], CFQ3[:, 0:NLON])
                nc.sync.dma_start(y_part[0:128, :], dbg[:, :])
                return nc

            # ---- stage C: channel mix, block-diag built on-chip, 2 MMs/l ----
            COUT4 = bigp.tile([64, 66 * 256], BF16, tag="bigA")
            COUT4_v = COUT4.rearrange("p (cm l) -> p cm l", l=256)
            for ci in range(8):
                wcT = wcp.tile([128, 16, 2, 64], BF16, tag="wc")
                nc.sync.dma_start(
                    wcT[:, :, :, :], wc.ap()[:, ci * 16:(ci + 1) * 16, :, :]
                )
                wt = wtp.tile([128, 16, 2, 128], BF16, tag="wt")
                if ci < 2:
                    nc.vector.memset(wt[:, :, :, :], 0.0)
                nc.vector.tensor_copy(wt[0:64, :, :, 0:64], wcT[0:64, :, :, :])
                nc.scalar.copy(wt[64:128, :, :, 64:128], wcT[64:128, :, :, :])
                for jj in range(0, 16, 2):
                    j = ci * 16 + jj
                    pc = psp.tile([128, 2, MC], F32, tag="ps")
                    for h in range(2):
                        nc.tensor.matmul(pc[:, h, :], wt[:, jj + h, 0, :],
                                         CFQ3_v[:, 33:99, j + h],
                                         start=(h == 0), stop=False)
                        nc.tensor.matmul(pc[:, h, :], wt[:, jj + h, 1, :],
                                         CFQ3_v[:, 0:66, j + h],
                                         start=False, stop=(h == 1))
                    d0 = COUT4_v[:, :, j:j + 2].rearrange("p cm h -> p h cm")
                    d1 = COUT4_v[:, :, 128 + j:130 + j].rearrange("p cm h -> p h cm")
                    if (j // 2) % 2 == 0:
                        nc.vector.tensor_copy(d0, pc[0:64, :, :])
                        nc.scalar.copy(d1, pc[64:128, :, :])
                    else:
                        nc.scalar.copy(d0, pc[0:64, :, :])
                        nc.vector.tensor_copy(d1, pc[64:128, :, :])

            if "P1" not in stages and "D" not in stages:
                dbg = ysbp.tile([64, NLON], BF16, tag="ys")
                nc.vector.tensor_copy(dbg[:, :], COUT4[:, 0:NLON])
                nc.sync.dma_start(y_part[0:64, :], dbg[:, :])
                return nc

            # ---- pivot P1: COUT4 -> OUTT [l2, (blk, mi, cp, o)] ----
            OUTT = bigp.tile([128, 2 * M_LOC * 2 * 64], BF16, tag="bigC")
            OUTT_v = OUTT.rearrange("p (blk mi cp o) -> p blk mi cp o",
                                    blk=2, cp=2, o=64)
            for cm in range(MC):
                cp, mi = cm // M_LOC, cm % M_LOC
                ptr = pstp.tile([128, 2, 64], BF16, tag="pst")
                for blk in range(2):
                    nc.tensor.transpose(
                        ptr[:, blk, :],
                        COUT4_v[:, cm, blk * 128:(blk + 1) * 128], isbb[:64, :64]
                    )
                dst = OUTT_v[:, :, mi, cp, :]
                if cm % 2 == 0:
                    nc.vector.tensor_copy(dst, ptr[:, :, :])
                else:
                    nc.scalar.copy(dst, ptr[:, :, :])

            if "D" not in stages:
                dbg = ysbp.tile([128, NLON], BF16, tag="ys")
                nc.vector.tensor_copy(dbg[:, :], OUTT[:, 0:NLON])
                nc.sync.dma_start(y_part[0:128, :], dbg[:, :])
                return nc

            # ---- stage D: parity inverse Legendre, k-fold in PE accum ----
            # XKS [128 ki, (kc 2, o 64, cm 66)]; kc=1 rows are k-reversed
            # pcs blocks: [0:128]=pct(blk0), [128:256]=pct(blk1), [256:384]=-pct(blk1)
            XKS = bigp.tile([128, 2 * 64 * MC], BF16, tag="bigA2")
            XKS_v = XKS.rearrange("p (kc o cm) -> p kc o cm", kc=2, o=64)
            for mib in range(0, M_LOC, 2):
                nm2 = min(2, M_LOC - mib)
                if mib % 4 == 0:
                    nmw = min(4, M_LOC - mib)
                    pcs = ptp.tile([128, 4, 384], BF16, tag="pt")
                    nc.sync.dma_start(
                        pcs[:, 0:nmw, :], pctb.ap()[:, mib:mib + nmw, :]
                    )
                pd = psp.tile([128, nm2, 2, 128], F32, tag="ps")
                for m2 in range(nm2):
                    mi = mib + m2
                    ow = (mib % 4) + m2
                    r0 = OUTT_v[:, 0, mi, :, :].rearrange("p cp o -> p (cp o)")
                    r1 = OUTT_v[:, 1, mi, :, :].rearrange("p cp o -> p (cp o)")
                    first = (m2 == 0)
                    last = (m2 == nm2 - 1)
                    # kc0 = blk0 + blk1 ; kc1 = blk0 - blk1
                    nc.tensor.matmul(pd[:, m2, 0, :], pcs[:, ow, 0:128], r0,
                                     start=first, stop=False)
                    nc.tensor.matmul(pd[:, m2, 0, :], pcs[:, ow, 128:256], r1,
                                     start=False, stop=False)
                    nc.tensor.matmul(pd[:, m2, 1, :], pcs[:, ow, 0:128], r0,
                                     start=False, stop=False)
                    nc.tensor.matmul(pd[:, m2, 1, :], pcs[:, ow, 256:384], r1,
                                     start=False, stop=last)
                for kc in range(2):
                    src = pd[:, :, kc, :].rearrange("p m2 (cp o) -> p m2 cp o",
                                                    cp=2, o=64)
                    dst = XKS_v[:, kc, :, 2 * mib:2 * (mib + nm2)] \
                        .rearrange("p o (m2 cp) -> p m2 cp o", cp=2)
                    if (mib // 2 + kc) % 2 == 0:
                        nc.vector.tensor_copy(dst, src)
                    else:
                        nc.scalar.copy(dst, src)

            if "P2" not in stages and "E" not in stages:
                dbg = ysbp.tile([128, NLON], BF16, tag="ys")
                nc.vector.tensor_copy(dbg[:, :], XKS[:, 0:NLON])
                nc.sync.dma_start(y_part[0:128, :], dbg[:, :])
                return nc

            # ---- pivot P2: XKS -> XK [(m,comp), (o, kpos)] ----
            XK = bigp.tile([MC, CK], BF16, tag="bigB2")
            XK_v = XK.rearrange("p (o k) -> p o k", k=NLAT)
            for o in range(64):
                pt2 = pstp.tile([MC, 2, 128], BF16, tag="pst")
                for kc in range(2):
                    nc.tensor.transpose(
                        pt2[:, kc, :], XKS_v[:, kc, o, :], isbb[:, :]
                    )
                if o % 2 == 0:
                    nc.vector.tensor_copy(XK_v[:, o, :], pt2.rearrange("p a b -> p (a b)"))
                else:
                    nc.scalar.copy(XK_v[:, o, :], pt2.rearrange("p a b -> p (a b)"))

            if "E" not in stages:
                dbg = ysbp.tile([MC, NLON], BF16, tag="ys")
                nc.vector.tensor_copy(dbg[:, :], XK[0:MC, 0:NLON])
                nc.sync.dma_start(y_part[0:MC, :], dbg[:, :])
                return nc

            # ---- stage E: inverse DFT as matmul bf16, contract m-comps ----
            nmod = 3 if GP_PSUM else 2
            for jp in range(CK // 256):
                ys = ysbp.tile([128, 2, NLON], BF16, tag="ys2")
                for h in range(2):
                    j = 2 * jp + h
                    pe = psp.tile([128, NLON], F32, tag="ps")
                    nc.tensor.matmul(
                        pe[:, :], XK[:, j * 128:(j + 1) * 128], gsb[:, :],
                        start=True, stop=True,
                    )
                    sel = (2 * jp + h) % nmod
                    if sel == 0:
                        nc.scalar.copy(ys[:, h, :], pe[:, :])
                    elif sel == 1:
                        nc.vector.tensor_copy(ys[:, h, :], pe[:, :])
                    else:
                        nc.gpsimd.tensor_copy(ys[:, h, :], pe[:, :])
                nc.sync.dma_start(
                    y_part.ap()[jp * 256:(jp + 1) * 256, :]
                    .rearrange("(a p) n -> p a n", a=2),
                    ys[:, :, :],
                )

    return nc


def _get_nc(stages="ABCDE"):
    if stages not in _prog_cache:
        nc = _build_nc(stages)
        nc.compile()
        _prog_cache[stages] = nc
    return _prog_cache[stages]


def _core_ms(r):
    return [8 * j + r for j in range(M_LOC) if 8 * j + r < MMAX]


def make_in_maps(x, weight_r, weight_i, pct, sht_w):
    x = np.asarray(x, dtype=np.float32)
    wr = np.asarray(weight_r, dtype=np.float32)[0]          # [i, o, l]
    wi = np.asarray(weight_i, dtype=np.float32)[0]
    pct = np.asarray(pct, dtype=np.float32)                 # [m, l, k]
    sht_w = np.asarray(sht_w, dtype=np.float32)

    # xt[nc4, ni, ck], ck = c*256 + kidx; kidx>=128 holds k = 255-(kidx-128)
    x2 = x[0].reshape(CIN, NLAT, NLON)
    xr = np.concatenate([x2[:, :128], x2[:, 255:127:-1]], axis=1)
    xt = np.ascontiguousarray(
        xr.reshape(CK, NLON).T.reshape(4, 128, CK)
    ).astype(ml_dtypes.bfloat16)
    ident = np.eye(128, dtype=np.float32).astype(ml_dtypes.bfloat16)

    n = np.arange(NLON)
    j128 = np.arange(128)
    in_maps = []
    wc_cache = {}
    for r in range(N_CORES):
        ms = _core_ms(r)
        nm = len(ms)
        marr = np.array(ms)
        p_core = r % 2

        ang = 2.0 * np.pi * marr[None, :] * n[:, None] / NLON   # [n, nm]
        fdft = np.zeros((NLON, MC), np.float32)
        fdft[:, :nm] = (2.0 * np.pi / NLON) * np.cos(ang)
        fdft[:, M_LOC:M_LOC + nm] = -(2.0 * np.pi / NLON) * np.sin(ang)
        fdft = fdft.reshape(4, 128, MC)

        cmf = np.where((marr == 0) | (marr == NLON // 2), 1.0, 2.0)
        gdft = np.zeros((MC, NLON), np.float32)   # rows interleaved (m, comp)
        gdft[0:2 * nm:2, :] = cmf[:, None] * np.cos(ang.T)
        gdft[1:2 * nm:2, :] = -cmf[:, None] * np.sin(ang.T)

        l_p0 = 2 * j128 + p_core          # sigma+ l's
        l_p1 = 2 * j128 + 1 - p_core      # sigma- l's

        shtw_h = np.zeros((128, M_LOC, 256), np.float32)   # [ki, mi, (blk,j)]
        pct_h = np.zeros((128, M_LOC, 384), np.float32)    # [l2, mi, (b0,b1,-b1)]
        for mi, m in enumerate(ms):
            shtw_h[:, mi, 0:128] = sht_w[m][l_p0, :128].T
            shtw_h[:, mi, 128:256] = sht_w[m][l_p1, :128].T
            pct_h[:, mi, 0:128] = pct[m][l_p0, :128]
            pct_h[:, mi, 128:256] = pct[m][l_p1, :128]
            pct_h[:, mi, 256:384] = -pct[m][l_p1, :128]

        if p_core not in wc_cache:
            wca = np.zeros((128, 128, 2, 64), np.float32)
            wca[0:64, :, 0, :] = wr[:, :, l_p0].transpose(0, 2, 1)
            wca[64:128, :, 0, :] = wr[:, :, l_p1].transpose(0, 2, 1)
            wca[0:64, :, 1, :] = wi[:, :, l_p0].transpose(0, 2, 1)
            wca[64:128, :, 1, :] = wi[:, :, l_p1].transpose(0, 2, 1)
            wc_cache[p_core] = np.ascontiguousarray(wca).astype(ml_dtypes.bfloat16)

        in_maps.append({
            "xt": xt,
            "fdft": np.ascontiguousarray(fdft).astype(ml_dtypes.bfloat16),
            "shtw": shtw_h.astype(ml_dtypes.bfloat16),
            "wc": wc_cache[p_core],
            "pctb": pct_h.astype(ml_dtypes.bfloat16),
            "gdft": gdft.astype(ml_dtypes.bfloat16),
            "ident": ident,
        })
    return in_maps


def kernel(x, weight_r, weight_i, pct, sht_w):
    x_np = np.asarray(x)
    nc = _get_nc()
    in_maps = make_in_maps(x_np, weight_r, weight_i, pct, sht_w)
    try:
        res = run_bass_kernel_spmd(nc, in_maps, list(range(N_CORES)))
    except Exception:
        # transient NRT exec faults have been observed on the first run
        # after a NEFF load; one retry has always succeeded
        res = run_bass_kernel_spmd(nc, in_maps, list(range(N_CORES)))
    y = np.zeros((CK, NLON), np.float64)
    for r in range(N_CORES):
        y += np.asarray(res.results[r]["y_part"], dtype=np.float64)
    y4 = y.astype(np.float32).reshape(COUT_, 2, 128, NLON)
    yf = np.concatenate([y4[:, 0], y4[:, 1, ::-1]], axis=1)
    return (yf.reshape(1, COUT_, NLAT, NLON), x_np)


# revision 27
# speedup vs baseline: 1.4154x; 1.1221x over previous
"""Distributed spectral conv on S2 (SHT -> per-l complex channel mix -> ISHT)
for Trainium2, m-mode sharded across 8 NeuronCores (strided: core r gets
m = r, r+8, r+16, ... so every core's m-list has uniform parity).

v2 pipeline per core (33 m-slots, MC=66 real components):
  A2: DFT as matmul, x-chunks stationary -> psum [ck 128, cm 66] (k on
      partitions directly; host bakes k-reversal for the second k-half)
  F:  parity fold XE/XO = kh0 +- kh1 (psum-pair -> sbuf bf16)
  B:  parity Legendre, K=128 bf16: per m two matmuls (XE@shtw+, XO@shtw-)
  C:  per-l-pair channel mix, block-diag built on-chip from compact weights,
      2 matmuls per l using a negated-cfi copy in CFQ3
  P1: PE-transpose COUT4 -> OUTT [l2, (blk, m, cp, o)]
  D:  parity inverse Legendre + psum add/sub fold -> XKS (kc=1 k-reversed)
  P2: PE-transpose -> XK [(m,comp), (o,k')]
  E:  inverse DFT bf16 (contract m-comps) -> y_part [CK, NLON]
Host sums the 8 partial y outputs and un-reverses the second k-half.
"""
import numpy as np
import ml_dtypes

import concourse.bass as bass
import concourse.bacc as bacc
import concourse.mybir as mybir
from concourse import tile
from concourse._compat import get_trn_type
from concourse.bass_utils import run_bass_kernel_spmd

F32 = mybir.dt.float32
BF16 = mybir.dt.bfloat16
ADD = mybir.AluOpType.add
SUB = mybir.AluOpType.subtract

N_CORES = 8
M_LOC = 33            # m slots per core (core 0: 33 real m's, others 32+pad)
MC = 2 * M_LOC
CIN = 64
COUT_ = 64
NLAT = 256
NLON = 512
MMAX = 257
CK = COUT_ * NLAT

GP_PSUM = False       # TRN2: GPSIMD instructions cannot access PSUM

_prog_cache = {}


def _build_nc(stages="ABCDE"):
    nc = bacc.Bacc(get_trn_type() or "TRN2", target_bir_lowering=False, debug=False)

    xt = nc.dram_tensor("xt", [4, 128, CK], BF16, kind="ExternalInput")
    fdft = nc.dram_tensor("fdft", [4, 128, MC], BF16, kind="ExternalInput")
    shtw = nc.dram_tensor("shtw", [128, M_LOC, 256], BF16, kind="ExternalInput")
    wc = nc.dram_tensor("wc", [128, 128, 2, 64], BF16, kind="ExternalInput")
    pctb = nc.dram_tensor("pctb", [128, M_LOC, 384], BF16, kind="ExternalInput")
    gdft = nc.dram_tensor("gdft", [MC, NLON], BF16, kind="ExternalInput")
    ident = nc.dram_tensor("ident", [128, 128], BF16, kind="ExternalInput")
    y_part = nc.dram_tensor("y_part", [CK, NLON], BF16, kind="ExternalOutput")

    with tile.TileContext(nc) as tc:
        with tc.tile_pool(name="const", bufs=1) as constp, \
             tc.tile_pool(name="big", bufs=1) as bigp, \
             tc.tile_pool(name="xa", bufs=2) as xap, \
             tc.tile_pool(name="sw", bufs=3) as swp, \
             tc.tile_pool(name="wcp", bufs=2) as wcp, \
             tc.tile_pool(name="wt", bufs=2) as wtp, \
             tc.tile_pool(name="pt", bufs=3) as ptp, \
             tc.tile_pool(name="ysb", bufs=4) as ysbp, \
             tc.tile_pool(name="ps", bufs=4, space="PSUM") as psp, \
             tc.tile_pool(name="pst", bufs=4, space="PSUM") as pstp:

            xar0 = xap.tile([128, 4, 1024], BF16, tag="xar")
            nc.sync.dma_start(
                xar0[:, :, :],
                xt.ap()[:, :, 0:1024].rearrange("a b c -> b a c")
            )
            fsbr = constp.tile([128, 4, MC], BF16)      # [n_in_chunk, nchunk, cm]
            fsbrN = constp.tile([128, 4, MC], BF16)     # negated (for XO fold)
            gsb = constp.tile([MC, NLON], BF16)
            isbb = constp.tile([128, 128], BF16)
            nc.sync.dma_start(fsbr[:, :, :], fdft.ap().rearrange("a b c -> b a c"))
            nc.sync.dma_start(gsb[:, :], gdft[:, :])
            nc.sync.dma_start(isbb[:, :], ident[:, :])
            nc.vector.tensor_scalar_mul(fsbrN[:, :, :], fsbr[:, :, :], -1.0)

            # ---- stage A: DFT as matmul, basis stationary (big-N MMs), then
            # T1 PE-transpose per channel and SBUF parity fold -> XE/XO
            # XE/XO [128 ki, (comp, c, mi)]: the per-mi lhsT slice is a single
            # stride-33 free dim, and folds have contiguous inner mi runs.
            # XE = xf(k) + xf(255-k)  (kh1 k-reversal baked in xt on host)
            XE = bigp.tile([128, CIN * MC], BF16, tag="bigE")
            XO = bigp.tile([128, CIN * MC], BF16, tag="bigO")
            XE_v = XE.rearrange("p (comp c mi) -> p comp c mi", comp=2, mi=M_LOC)
            XO_v = XO.rearrange("p (comp c mi) -> p comp c mi", comp=2, mi=M_LOC)
            XE_b = XE.rearrange("p (cc mi) -> p cc mi", mi=M_LOC)
            XO_b = XO.rearrange("p (cc mi) -> p cc mi", mi=M_LOC)
            for span in range(16):          # 1024 ck-columns per span
                if span == 0:
                    xar = xar0
                else:
                    xar = xap.tile([128, 4, 1024], BF16, tag="xar")
                    nc.sync.dma_start(
                        xar[:, :, :],
                        xt.ap()[:, :, span * 1024:(span + 1) * 1024]
                        .rearrange("a b c -> b a c")
                    )
                for half in range(2):       # 512 ck-cols = 2 channels per psum
                    pa = psp.tile([MC, 512], F32, tag="ps")
                    for nc4 in range(4):
                        nc.tensor.matmul(
                            pa[:, :],
                            fsbr[:, nc4, :],
                            xar[:, nc4, half * 512:(half + 1) * 512],
                            start=(nc4 == 0),
                            stop=(nc4 == 3),
                        )
                    xf2 = xap.tile([MC, 512], BF16, tag="xf2")
                    nc.scalar.copy(xf2[:, :], pa[:, :])
                    for cc in range(2):     # channel c0+cc: q-blocks (kh0,kh1)
                        c = span * 4 + half * 2 + cc
                        ptr = pstp.tile([128, 2, MC], BF16, tag="pst")
                        for kh in range(2):
                            nc.tensor.transpose(
                                ptr[:, kh, :],
                                xf2[:, (2 * cc + kh) * 128:(2 * cc + kh + 1) * 128],
                                isbb[:MC, :MC]
                            )
                        xfc = xap.tile([128, 2, MC], BF16, tag="xfc")
                        if cc == 0:
                            nc.vector.tensor_copy(xfc[:, :, :], ptr[:, :, :])
                        else:
                            nc.scalar.copy(xfc[:, :, :], ptr[:, :, :])
                        sf0 = xfc[:, 0, :].rearrange("p (comp mi) -> p comp mi", comp=2)
                        sf1 = xfc[:, 1, :].rearrange("p (comp mi) -> p comp mi", comp=2)
                        nc.vector.tensor_tensor(
                            XE_v[:, :, c, :], sf0, sf1, op=ADD)
                        nc.vector.tensor_tensor(
                            XO_v[:, :, c, :], sf0, sf1, op=SUB)

            if "B" not in stages:
                dbg = ysbp.tile([128, NLON], BF16, tag="ys")
                nc.vector.tensor_copy(dbg[:, :], XE[:, 0:NLON])
                nc.sync.dma_start(y_part[0:128, :], dbg[:, :])
                nc.vector.tensor_copy(dbg[:, :], XO[:, 0:NLON])
                nc.sync.dma_start(y_part[128:256, :], dbg[:, :])
                return nc

            # ---- stage B: parity Legendre bf16 -> CFQ3 ----
            # CFQ3 [(blk,i) 128, (cm3, j)]; cm3: [0:33]=-cfi, [33:66]=cfr, [66:99]=cfi
            CFQ3 = bigp.tile([128, 99 * 128], BF16, tag="bigB")
            CFQ3_v = CFQ3.rearrange("p (cm j) -> p cm j", j=128)
            for mib in range(0, M_LOC, 2):
                nm2 = min(2, M_LOC - mib)
                if mib % 4 == 0:
                    nmw = min(4, M_LOC - mib)
                    swr = swp.tile([128, 4, 256], BF16, tag="sw")
                    nc.sync.dma_start(
                        swr[:, 0:nmw, :], shtw.ap()[:, mib:mib + nmw, :]
                    )
                pb = psp.tile([128, nm2 * 256], F32, tag="ps")
                pb_v = pb.rearrange("p (m2 blk j) -> p m2 blk j", blk=2, j=128)
                for m2 in range(nm2):
                    mi = mib + m2
                    nc.tensor.matmul(
                        pb_v[:, m2, 0, :],
                        XE_b[:, :, mi],
                        swr[:, (mib % 4) + m2, 0:128],
                        start=(m2 == 0), stop=False,
                    )
                    nc.tensor.matmul(
                        pb_v[:, m2, 1, :],
                        XO_b[:, :, mi],
                        swr[:, (mib % 4) + m2, 128:256],
                        start=False, stop=(m2 == nm2 - 1),
                    )
                for blk in range(2):
                    for comp in range(2):
                        dst = CFQ3_v[blk * 64:(blk + 1) * 64,
                                     (33 if comp == 0 else 66) + mib:
                                     (33 if comp == 0 else 66) + mib + nm2, :]
                        src = pb_v[comp * 64:(comp + 1) * 64, :, blk, :]
                        if (mib // 2 + blk) % 2 == 0:
                            nc.vector.tensor_copy(dst, src)
                        else:
                            nc.scalar.copy(dst, src)
            # bulk negate: cm3[0:33] = -cfi
            nc.vector.tensor_scalar_mul(
                CFQ3_v[:, 0:33, :], CFQ3_v[:, 66:99, :], -1.0
            )

            if "C" not in stages:
                dbg = ysbp.tile([128, NLON], BF16, tag="ys")
                nc.vector.tensor_copy(dbg[:, :], CFQ3[:, 0:NLON])
                nc.sync.dma_start(y_part[0:128, :], dbg[:, :])
                return nc

            # ---- stage C: channel mix, block-diag built on-chip, 2 MMs/l ----
            # COUT4 [64 o, (lpos 256, cm 66)]: lpos-major so evac is contiguous
            COUT4 = bigp.tile([64, 256 * 66], BF16, tag="bigA")
            COUT4_v = COUT4.rearrange("p (l cm) -> p l cm", cm=66)
            for ci in range(8):
                wcT = wcp.tile([128, 16, 2, 64], BF16, tag="wc")
                nc.sync.dma_start(
                    wcT[:, :, :, :], wc.ap()[:, ci * 16:(ci + 1) * 16, :, :]
                )
                wt = wtp.tile([128, 16, 2, 128], BF16, tag="wt")
                if ci < 2:
                    nc.vector.memset(wt[:, :, :, :], 0.0)
                nc.vector.tensor_copy(wt[0:64, :, :, 0:64], wcT[0:64, :, :, :])
                nc.scalar.copy(wt[64:128, :, :, 64:128], wcT[64:128, :, :, :])
                for jj in range(0, 16, 2):
                    j = ci * 16 + jj
                    pc = psp.tile([128, 2, MC], F32, tag="ps")
                    for h in range(2):
                        nc.tensor.matmul(pc[:, h, :], wt[:, jj + h, 0, :],
                                         CFQ3_v[:, 33:99, j + h],
                                         start=(h == 0), stop=False)
                        nc.tensor.matmul(pc[:, h, :], wt[:, jj + h, 1, :],
                                         CFQ3_v[:, 0:66, j + h],
                                         start=False, stop=(h == 1))
                    d0 = COUT4_v[:, j:j + 2, :]
                    d1 = COUT4_v[:, 128 + j:130 + j, :]
                    if (j // 2) % 2 == 0:
                        nc.vector.tensor_copy(d0, pc[0:64, :, :])
                        nc.scalar.copy(d1, pc[64:128, :, :])
                    else:
                        nc.scalar.copy(d0, pc[0:64, :, :])
                        nc.vector.tensor_copy(d1, pc[64:128, :, :])

            if "P1" not in stages and "D" not in stages:
                dbg = ysbp.tile([64, NLON], BF16, tag="ys")
                nc.vector.tensor_copy(dbg[:, :], COUT4[:, 0:NLON])
                nc.sync.dma_start(y_part[0:64, :], dbg[:, :])
                return nc

            # ---- pivot P1: COUT4 -> OUTT [l2, (blk, mi, cp, o)] ----
            OUTT = bigp.tile([128, 2 * M_LOC * 2 * 64], BF16, tag="bigC")
            OUTT_v = OUTT.rearrange("p (blk mi cp o) -> p blk mi cp o",
                                    blk=2, cp=2, o=64)
            for cm in range(MC):
                cp, mi = cm // M_LOC, cm % M_LOC
                ptr = pstp.tile([128, 2, 64], BF16, tag="pst")
                for blk in range(2):
                    nc.tensor.transpose(
                        ptr[:, blk, :],
                        COUT4_v[:, blk * 128:(blk + 1) * 128, cm], isbb[:64, :64]
                    )
                dst = OUTT_v[:, :, mi, cp, :]
                if cm % 2 == 0:
                    nc.vector.tensor_copy(dst, ptr[:, :, :])
                else:
                    nc.scalar.copy(dst, ptr[:, :, :])

            if "D" not in stages:
                dbg = ysbp.tile([128, NLON], BF16, tag="ys")
                nc.vector.tensor_copy(dbg[:, :], OUTT[:, 0:NLON])
                nc.sync.dma_start(y_part[0:128, :], dbg[:, :])
                return nc

            # ---- stage D: parity inverse Legendre, k-fold in PE accum ----
            # XKS [128 ki, (kc 2, o 64, cm 66)]; kc=1 rows are k-reversed
            # pcs blocks: [0:128]=pct(blk0), [128:256]=pct(blk1), [256:384]=-pct(blk1)
            XKS = bigp.tile([128, 2 * MC * 64], BF16, tag="bigA2")
            XKS_v = XKS.rearrange("p (kc cm o) -> p kc cm o", kc=2, o=64)
            for mib in range(0, M_LOC, 2):
                nm2 = min(2, M_LOC - mib)
                if mib % 4 == 0:
                    nmw = min(4, M_LOC - mib)
                    pcs = ptp.tile([128, 4, 384], BF16, tag="pt")
                    nc.sync.dma_start(
                        pcs[:, 0:nmw, :], pctb.ap()[:, mib:mib + nmw, :]
                    )
                pd = psp.tile([128, nm2, 2, 128], F32, tag="ps")
                for m2 in range(nm2):
                    mi = mib + m2
                    ow = (mib % 4) + m2
                    r0 = OUTT_v[:, 0, mi, :, :].rearrange("p cp o -> p (cp o)")
                    r1 = OUTT_v[:, 1, mi, :, :].rearrange("p cp o -> p (cp o)")
                    first = (m2 == 0)
                    last = (m2 == nm2 - 1)
                    # kc0 = blk0 + blk1 ; kc1 = blk0 - blk1
                    nc.tensor.matmul(pd[:, m2, 0, :], pcs[:, ow, 0:128], r0,
                                     start=first, stop=False)
                    nc.tensor.matmul(pd[:, m2, 0, :], pcs[:, ow, 128:256], r1,
                                     start=False, stop=False)
                    nc.tensor.matmul(pd[:, m2, 1, :], pcs[:, ow, 0:128], r0,
                                     start=False, stop=False)
                    nc.tensor.matmul(pd[:, m2, 1, :], pcs[:, ow, 256:384], r1,
                                     start=False, stop=last)
                for kc in range(2):
                    src = pd[:, :, kc, :]
                    dst = XKS_v[:, kc, 2 * mib:2 * (mib + nm2), :] \
                        .rearrange("p (m2 cp) o -> p m2 (cp o)", cp=2)
                    if (mib // 2 + kc) % 2 == 0:
                        nc.vector.tensor_copy(dst, src)
                    else:
                        nc.scalar.copy(dst, src)

            if "P2" not in stages and "E" not in stages:
                dbg = ysbp.tile([128, NLON], BF16, tag="ys")
                nc.vector.tensor_copy(dbg[:, :], XKS[:, 0:NLON])
                nc.sync.dma_start(y_part[0:128, :], dbg[:, :])
                return nc

            # ---- pivot P2: XKS -> XK [(m,comp), (o, kpos)] ----
            XK = bigp.tile([MC, CK], BF16, tag="bigB2")
            XK_v = XK.rearrange("p (o k) -> p o k", k=NLAT)
            for o in range(64):
                pt2 = pstp.tile([MC, 2, 128], BF16, tag="pst")
                for kc in range(2):
                    nc.tensor.transpose(
                        pt2[:, kc, :], XKS_v[:, kc, :, o], isbb[:, :]
                    )
                if o % 2 == 0:
                    nc.vector.tensor_copy(XK_v[:, o, :], pt2.rearrange("p a b -> p (a b)"))
                else:
                    nc.scalar.copy(XK_v[:, o, :], pt2.rearrange("p a b -> p (a b)"))

            if "E" not in stages:
                dbg = ysbp.tile([MC, NLON], BF16, tag="ys")
                nc.vector.tensor_copy(dbg[:, :], XK[0:MC, 0:NLON])
                nc.sync.dma_start(y_part[0:MC, :], dbg[:, :])
                return nc

            # ---- stage E: inverse DFT as matmul bf16, contract m-comps ----
            nmod = 3 if GP_PSUM else 2
            for jp in range(CK // 256):
                ys = ysbp.tile([128, 2, NLON], BF16, tag="ys2")
                for h in range(2):
                    j = 2 * jp + h
                    pe = psp.tile([128, NLON], F32, tag="ps")
                    nc.tensor.matmul(
                        pe[:, :], XK[:, j * 128:(j + 1) * 128], gsb[:, :],
                        start=True, stop=True,
                    )
                    sel = (2 * jp + h) % nmod
                    if sel == 0:
                        nc.scalar.copy(ys[:, h, :], pe[:, :])
                    elif sel == 1:
                        nc.vector.tensor_copy(ys[:, h, :], pe[:, :])
                    else:
                        nc.gpsimd.tensor_copy(ys[:, h, :], pe[:, :])
                nc.sync.dma_start(
                    y_part.ap()[jp * 256:(jp + 1) * 256, :]
                    .rearrange("(a p) n -> p a n", a=2),
                    ys[:, :, :],
                )

    return nc


def _get_nc(stages="ABCDE"):
    if stages not in _prog_cache:
        nc = _build_nc(stages)
        nc.compile()
        _prog_cache[stages] = nc
    return _prog_cache[stages]


def _core_ms(r):
    return [8 * j + r for j in range(M_LOC) if 8 * j + r < MMAX]


def make_in_maps(x, weight_r, weight_i, pct, sht_w):
    x = np.asarray(x, dtype=np.float32)
    wr = np.asarray(weight_r, dtype=np.float32)[0]          # [i, o, l]
    wi = np.asarray(weight_i, dtype=np.float32)[0]
    pct = np.asarray(pct, dtype=np.float32)                 # [m, l, k]
    sht_w = np.asarray(sht_w, dtype=np.float32)

    # xt[nc4, ni, ck], ck = c*256 + kidx; kidx>=128 holds k = 255-(kidx-128)
    x2 = x[0].reshape(CIN, NLAT, NLON)
    xr = np.concatenate([x2[:, :128], x2[:, 255:127:-1]], axis=1)
    xt = np.ascontiguousarray(
        xr.reshape(CK, NLON).T.reshape(4, 128, CK)
    ).astype(ml_dtypes.bfloat16)
    ident = np.eye(128, dtype=np.float32).astype(ml_dtypes.bfloat16)

    n = np.arange(NLON)
    j128 = np.arange(128)
    in_maps = []
    wc_cache = {}
    for r in range(N_CORES):
        ms = _core_ms(r)
        nm = len(ms)
        marr = np.array(ms)
        p_core = r % 2

        ang = 2.0 * np.pi * marr[None, :] * n[:, None] / NLON   # [n, nm]
        fdft = np.zeros((NLON, MC), np.float32)
        fdft[:, :nm] = (2.0 * np.pi / NLON) * np.cos(ang)
        fdft[:, M_LOC:M_LOC + nm] = -(2.0 * np.pi / NLON) * np.sin(ang)
        fdft = fdft.reshape(4, 128, MC)

        cmf = np.where((marr == 0) | (marr == NLON // 2), 1.0, 2.0)
        gdft = np.zeros((MC, NLON), np.float32)   # rows interleaved (m, comp)
        gdft[0:2 * nm:2, :] = cmf[:, None] * np.cos(ang.T)
        gdft[1:2 * nm:2, :] = -cmf[:, None] * np.sin(ang.T)

        l_p0 = 2 * j128 + p_core          # sigma+ l's
        l_p1 = 2 * j128 + 1 - p_core      # sigma- l's

        shtw_h = np.zeros((128, M_LOC, 256), np.float32)   # [ki, mi, (blk,j)]
        pct_h = np.zeros((128, M_LOC, 384), np.float32)    # [l2, mi, (b0,b1,-b1)]
        for mi, m in enumerate(ms):
            shtw_h[:, mi, 0:128] = sht_w[m][l_p0, :128].T
            shtw_h[:, mi, 128:256] = sht_w[m][l_p1, :128].T
            pct_h[:, mi, 0:128] = pct[m][l_p0, :128]
            pct_h[:, mi, 128:256] = pct[m][l_p1, :128]
            pct_h[:, mi, 256:384] = -pct[m][l_p1, :128]

        if p_core not in wc_cache:
            wca = np.zeros((128, 128, 2, 64), np.float32)
            wca[0:64, :, 0, :] = wr[:, :, l_p0].transpose(0, 2, 1)
            wca[64:128, :, 0, :] = wr[:, :, l_p1].transpose(0, 2, 1)
            wca[0:64, :, 1, :] = wi[:, :, l_p0].transpose(0, 2, 1)
            wca[64:128, :, 1, :] = wi[:, :, l_p1].transpose(0, 2, 1)
            wc_cache[p_core] = np.ascontiguousarray(wca).astype(ml_dtypes.bfloat16)

        in_maps.append({
            "xt": xt,
            "fdft": np.ascontiguousarray(fdft).astype(ml_dtypes.bfloat16),
            "shtw": shtw_h.astype(ml_dtypes.bfloat16),
            "wc": wc_cache[p_core],
            "pctb": pct_h.astype(ml_dtypes.bfloat16),
            "gdft": gdft.astype(ml_dtypes.bfloat16),
            "ident": ident,
        })
    return in_maps


def kernel(x, weight_r, weight_i, pct, sht_w):
    x_np = np.asarray(x)
    nc = _get_nc()
    in_maps = make_in_maps(x_np, weight_r, weight_i, pct, sht_w)
    try:
        res = run_bass_kernel_spmd(nc, in_maps, list(range(N_CORES)))
    except Exception:
        # transient NRT exec faults have been observed on the first run
        # after a NEFF load; one retry has always succeeded
        res = run_bass_kernel_spmd(nc, in_maps, list(range(N_CORES)))
    y = np.zeros((CK, NLON), np.float64)
    for r in range(N_CORES):
        y += np.asarray(res.results[r]["y_part"], dtype=np.float64)
    y4 = y.astype(np.float32).reshape(COUT_, 2, 128, NLON)
    yf = np.concatenate([y4[:, 0], y4[:, 1, ::-1]], axis=1)
    return (yf.reshape(1, COUT_, NLAT, NLON), x_np)
